# revision 30
# baseline (speedup 1.0000x reference)
"""CrossAttention Trainium2 kernel.

Shapes (hardcoded from the problem spec):
  x  (32, 1024, 512) f32, xf (32, 77, 256) f32
  ln_g/ln_b (512,), tln_g/tln_b (256,)
  Wq (512,512), Wk (256,512), Wv (256,512), bq/bk/bv (512,)
  out y (32, 1024, 512) f32

Strategy:
  - Data-parallel over batch: 32 batches -> 8 cores x 4 batches. No collectives.
  - Host folds LayerNorm gamma/beta and biases into the projection weights
    (constant folding of parameters only), casts x/xf/weights to bf16.
  - x^T is loaded DIRECTLY from DRAM via the DMA xbar transpose; the x
    LayerNorm affine is deferred past the Q matmul:
      qraw^T = Wq'^T @ x^T  -  wqsum (x) mean   (mean fix = 5th accumulating
               matmul with a single-row lhsT of -colsum(Wq'))
      q^T    = qraw^T * rB                      (rB = per-column rstd
               broadcast tile built by a tiny PE outer product)
    Stats (mean/rstd rows) come from a natural-layout x load via bn_stats;
    they are reshaped to [1, T] rows with one small PE transpose + SBUF DMA.
  - Device per batch:
      xf: bn_stats layernorm -> PE transpose -> K^T and [V|1] projections
      S^T per head pair row-packed (tile_position) into a 2-bank PSUM tile,
      ONE Exp ACT per (head pair, T half) over [77,1024]
      y = P^T.T @ [V|1], softmax denominator in column 64; normalize on
      vector; y out per T-half on gpsimd.
"""

import numpy as np
import ml_dtypes

import concourse.bass as bass
import concourse.bacc as bacc
import concourse.mybir as mybir
import concourse.tile as tile
from concourse.bass_utils import run_bass_kernel_spmd
from concourse.masks import make_identity

B, T, D, N, L, H = 32, 1024, 512, 77, 256, 8
HD = D // H           # 64
NCORES = 8
BPC = B // NCORES     # 4 batches per core
EPS = 1e-5
SCALE = 1.0 / np.sqrt(HD)  # 0.125

BF16 = mybir.dt.bfloat16
F32 = mybir.dt.float32

TC = T // 128         # 8 T-chunks per batch
DC = D // 128         # 4 D-chunks
LC = L // 128         # 2 L-chunks


class _Bacc(bacc.Bacc):
    """Bacc whose ACT-table chooser only finds Exp/Ln in the combined
    natural_log_exp_and_others set, so the kernel needs one table load
    instead of ping-ponging between exp_and_others and the ln set."""

    def insert_act_table_loads(self):
        import bass_rust as _br
        from concourse.hw_specs import get_activation_tables

        has_activation = any(
            isinstance(i, mybir.InstActivation)
            for blk in self.main_func.blocks
            for i in blk.instructions
        )
        if not has_activation:
            return
        pair = {
            mybir.ActivationFunctionType.Exp,
            mybir.ActivationFunctionType.Ln,
        }
        tables = []
        for name, fns in get_activation_tables(self.m.arch).items():
            if name != "natural_log_exp_and_others":
                fns = fns - pair
            tables.append((name, fns))
        _br.insert_act_table_loads(self, tables)


def _build(bpc=BPC, has_cq=False, has_ck=False, has_cv=False):
    nc = _Bacc("TRN2", target_bir_lowering=False, debug=False)

    xh = nc.dram_tensor("xh", (bpc, T, D), BF16, kind="ExternalInput")
    xfh = nc.dram_tensor("xfh", (bpc, N, L), BF16, kind="ExternalInput")
    wq = nc.dram_tensor("wq", (D, D), BF16, kind="ExternalInput")
    wk = nc.dram_tensor("wk", (L, D), BF16, kind="ExternalInput")
    wv = nc.dram_tensor("wv", (L, D), BF16, kind="ExternalInput")
    nwqs = nc.dram_tensor("nwqs", (1, D), BF16, kind="ExternalInput")
    cq_d = nc.dram_tensor("cq", (1, D), F32, kind="ExternalInput") if has_cq else None
    ck_d = nc.dram_tensor("ck", (1, D), F32, kind="ExternalInput") if has_ck else None
    cv_d = nc.dram_tensor("cv", (1, D), BF16, kind="ExternalInput") if has_cv else None
    y = nc.dram_tensor("y", (bpc, T, D), BF16, kind="ExternalOutput")

    with tile.TileContext(nc) as tc:
        _trace(tc, bpc, xh, xfh, wq, wk, wv, nwqs, cq_d, ck_d, cv_d, y)
    nc.compile()
    return nc


def _trace(tc, bpc, xh, xfh, wq, wk, wv, nwqs, cq_d, ck_d, cv_d, y):
    nc = tc.nc
    from contextlib import ExitStack

    ctx = ExitStack()
    with ctx:
        consts = ctx.enter_context(tc.tile_pool(name="consts", bufs=1))
        dramp = ctx.enter_context(tc.tile_pool(name="dramp", bufs=3, space="DRAM"))
        xpool = ctx.enter_context(tc.tile_pool(name="xpool", bufs=4))
        stats = ctx.enter_context(tc.tile_pool(name="stats", bufs=10))
        rowsp = ctx.enter_context(tc.tile_pool(name="rowsp", bufs=3))
        rbpool = ctx.enter_context(tc.tile_pool(name="rbpool", bufs=3))
        xfpool = ctx.enter_context(tc.tile_pool(name="xfpool", bufs=6))
        kvpool = ctx.enter_context(tc.tile_pool(name="kvpool", bufs=6))
        xtpool = ctx.enter_context(tc.tile_pool(name="xtpool", bufs=4))
        qpool = ctx.enter_context(tc.tile_pool(name="qpool", bufs=2))
        ptpool = ctx.enter_context(tc.tile_pool(name="ptpool", bufs=10))
        ypool = ctx.enter_context(tc.tile_pool(name="ypool", bufs=4))
        # PSUM pools: 8 banks total. fq 2x1 + st 2x2 + yps 2x1 = 8.
        fq = ctx.enter_context(tc.tile_pool(name="fq", bufs=2, space="PSUM"))
        stp = ctx.enter_context(tc.tile_pool(name="stp", bufs=2, space="PSUM"))
        yps = ctx.enter_context(tc.tile_pool(name="yps", bufs=2, space="PSUM"))

        # ---- batch-0 x loads first: they gate stats/Q and must lead the
        # scalar (natural) and sync (xbar) DMA rings ----
        preload = {}
        x_ts0 = []
        for hf in range(2):
            x_t = xpool.tile([128, 4, D], BF16, tag="x", name="x_t")
            for q in range(2):
                nc.scalar.dma_start(
                    out=x_t[:, 2 * q:2 * q + 2, :],
                    in_=xh[0, hf * 512 + q * 256:hf * 512 + (q + 1) * 256]
                    .rearrange("(c p) d -> p c d", p=128),
                )
            x_ts0.append(x_t)
        preload["x_ts"] = x_ts0
        xT0 = xtpool.tile([128, DC, T], BF16, tag="xT", name="xT")
        nc.sync.dma_start(out=xT0, in_=xh[0], transpose=True)
        preload["xT"] = xT0

        # ---- constants ----
        wk_sb = consts.tile([128, LC, D], BF16, tag="wk")
        nc.scalar.dma_start(out=wk_sb, in_=wk.rearrange("(c p) d -> p c d", p=128))
        wv_sb = consts.tile([128, LC, D], BF16, tag="wv")
        nc.scalar.dma_start(out=wv_sb, in_=wv.rearrange("(c p) d -> p c d", p=128))
        wq_sb = consts.tile([128, DC, D], BF16, tag="wq")
        nc.scalar.dma_start(out=wq_sb, in_=wq.rearrange("(c p) d -> p c d", p=128))
        nwqs_sb = consts.tile([1, D], BF16, tag="nwqs")
        nc.gpsimd.dma_start(out=nwqs_sb, in_=nwqs[:, :])
        eps_t = consts.tile([128, 1], F32, tag="eps")
        nc.vector.memset(eps_t, EPS)
        ident = consts.tile([128, 128], BF16, tag="ident")
        make_identity(nc, ident)
        ones1 = consts.tile([1, 128], BF16, tag="ones1")
        nc.vector.memset(ones1, 1.0)
        cq_sb = ck_sb = cv_sb = None
        if cq_d is not None:
            cq_sb = consts.tile([128, DC], F32, tag="cq")  # [dout_part, chunk]
            nc.gpsimd.dma_start(
                out=cq_sb, in_=cq_d.rearrange("o (c p) -> (o p) c", p=128)
            )
        if ck_d is not None:
            ck_sb = consts.tile([128, DC], F32, tag="ck")
            nc.gpsimd.dma_start(
                out=ck_sb, in_=ck_d.rearrange("o (c p) -> (o p) c", p=128)
            )
        if cv_d is not None:
            cv_sb = consts.tile([1, D], BF16, tag="cv")
            nc.gpsimd.dma_start(out=cv_sb, in_=cv_d)
            ones_row = consts.tile([1, N], BF16, tag="ones_row")
            nc.vector.memset(ones_row, 1.0)

        kT_b, vt_b, xT_b, rows_b, rB_b = {}, {}, {}, {}, {}

        def stage_a(b):
            """xf path (K^T, [V|1]) + x^T xbar loads + x stats -> r/m rows."""
            # natural x load first (it gates stats -> rows -> everything);
            # b0 split in quarters on sync for fastest stats start.
            if b == 0:
                x_ts = preload["x_ts"]
                xT = preload["xT"]
            else:
                x_ts = []
                for hf in range(2):
                    x_t = xpool.tile([128, 4, D], BF16, tag="x")
                    nc.gpsimd.dma_start(
                        out=x_t,
                        in_=xh[b, hf * 512:(hf + 1) * 512].rearrange(
                            "(c p) d -> p c d", p=128
                        ),
                    )
                    x_ts.append(x_t)
                # transposed x load: single whole-tensor xbar DMA; lands as
                # xT[p, dc, t] = x[t, dc*128+p] (full-row DRAM reads)
                xT = xtpool.tile([128, DC, T], BF16, tag="xT", name="xT")
                nc.sync.dma_start(out=xT, in_=xh[b], transpose=True)
            xT_b[b] = xT

            xf_t = xfpool.tile([N, L], BF16, tag="xf")
            nc.gpsimd.dma_start(out=xf_t, in_=xfh[b])
            st6 = stats.tile([N, 6], F32, tag="fst6")
            nc.vector.bn_stats(out=st6, in_=xf_t)
            mv_f = stats.tile([N, 2], F32, tag="fmv")
            nc.vector.bn_aggr(out=mv_f, in_=st6)
            # rstd = exp(-0.5*ln(var+eps)): Ln/Exp share one ACT table set.
            rstd_f = stats.tile([N, 1], F32, tag="frstd")
            nc.scalar.activation(
                out=rstd_f, in_=mv_f[:, 1:2],
                func=mybir.ActivationFunctionType.Ln,
                bias=eps_t[:N], scale=1.0,
            )
            nc.scalar.activation(
                out=rstd_f, in_=rstd_f,
                func=mybir.ActivationFunctionType.Exp, scale=-0.5,
            )
            xfn = xfpool.tile([N, L], BF16, tag="xfn")
            nc.vector.tensor_scalar(
                out=xfn, in0=xf_t,
                scalar1=mv_f[:, 0:1], scalar2=rstd_f,
                op0=mybir.AluOpType.subtract, op1=mybir.AluOpType.mult,
            )
            xfnT = xfpool.tile([128, LC, N], BF16, tag="xfnT")
            for c in range(LC):
                tps = fq.tile([128, N], BF16, tag="fq")
                nc.tensor.transpose(
                    out=tps, in_=xfn[:, c * 128:(c + 1) * 128], identity=ident[:N, :N]
                )
                nc.vector.tensor_copy(out=xfnT[:, c, :], in_=tps)

            kT = kvpool.tile([128, DC, N], BF16, tag="kT")
            for dc in range(DC):
                kps = fq.tile([128, N], F32, tag="fq")
                for lc in range(LC):
                    nc.tensor.matmul(
                        kps,
                        lhsT=wk_sb[:, lc, dc * 128:(dc + 1) * 128],
                        rhs=xfnT[:, lc, :],
                        start=(lc == 0), stop=(lc == LC - 1),
                    )
                if ck_sb is not None:
                    nc.vector.tensor_scalar_add(
                        out=kps, in0=kps, scalar1=ck_sb[:, dc:dc + 1]
                    )
                nc.scalar.copy(out=kT[:, dc, :], in_=kps)
            kT_b[b] = kT

            vps = fq.tile([N, D], F32, tag="fq")
            for lc in range(LC):
                nc.tensor.matmul(
                    vps, lhsT=xfnT[:, lc, :], rhs=wv_sb[:, lc, :],
                    start=(lc == 0), stop=(lc == LC - 1 and cv_sb is None),
                )
            if cv_sb is not None:
                nc.tensor.matmul(vps, lhsT=ones_row, rhs=cv_sb, start=False, stop=True)
            vt = kvpool.tile([N, H, HD + 1], BF16, tag="vt")
            nc.vector.tensor_copy(
                out=vt[:, :, 0:HD], in_=vps.rearrange("n (h d) -> n h d", h=H)
            )
            nc.vector.memset(vt[:, :, HD:HD + 1], 1.0)
            vt_b[b] = vt

            # x stats.  rt_in columns: 0..7 = rstd per (hf, c) t-chunk,
            # 8..15 = mean per (hf, c).
            rt_in = stats.tile([128, 16], BF16, tag="rt_in")
            for hf in range(2):
                x_t = x_ts[hf]
                mv4 = stats.tile([128, 4, 2], F32, tag="mv4")
                for c in range(4):
                    s6 = stats.tile([128, 6], F32, tag="xst6")
                    nc.vector.bn_stats(out=s6, in_=x_t[:, c, :])
                    nc.vector.bn_aggr(out=mv4[:, c, :], in_=s6)
                nc.scalar.activation(
                    out=rt_in[:, hf * 4:(hf + 1) * 4], in_=mv4[:, :, 1:2],
                    func=mybir.ActivationFunctionType.Ln,
                    bias=eps_t, scale=1.0,
                )
                nc.scalar.activation(
                    out=rt_in[:, hf * 4:(hf + 1) * 4],
                    in_=rt_in[:, hf * 4:(hf + 1) * 4],
                    func=mybir.ActivationFunctionType.Exp, scale=-0.5,
                )
                nc.vector.tensor_copy(
                    out=rt_in[:, 8 + hf * 4:8 + (hf + 1) * 4], in_=mv4[:, :, 0]
                )
            # [128,16] -> [16,128] -> bf16 -> [2,1024] rows (r_row, m_row)
            rt_ps = fq.tile([16, 128], BF16, tag="fq")
            nc.tensor.transpose(out=rt_ps, in_=rt_in, identity=ident)
            rt_sb = rowsp.tile([16, 128], BF16, tag="rt_sb")
            nc.scalar.copy(out=rt_sb, in_=rt_ps)
            # partition->free reshape via a small DRAM bounce (SBUF-to-SBUF
            # DMA cannot retile partitions)
            rt_d = dramp.tile([16, 128], BF16, tag="rt_d")
            nc.scalar.dma_start(out=rt_d, in_=rt_sb)
            rrow_t = rowsp.tile([1, 8, 128], BF16, tag="rrow_t")
            nc.scalar.dma_start(
                out=rrow_t, in_=rt_d[0:8].rearrange("(j k) f -> j k f", j=1)
            )
            mrow_t = rowsp.tile([1, 8, 128], BF16, tag="mrow_t")
            nc.scalar.dma_start(
                out=mrow_t, in_=rt_d[8:16].rearrange("(j k) f -> j k f", j=1)
            )
            rows_b[b] = (rrow_t, mrow_t)

            # rB[p, hf, t] = rstd[t] broadcast down partitions (outer product)
            rB = rbpool.tile([128, 2, 512], BF16, tag="rB")
            for hf in range(2):
                rbp = fq.tile([128, 512], F32, tag="fq")
                nc.tensor.matmul(
                    rbp, lhsT=ones1,
                    rhs=rows_b[b][0][:, hf * 4:(hf + 1) * 4, :],
                    start=True, stop=True,
                )
                nc.scalar.copy(out=rB[:, hf, :], in_=rbp)
            rB_b[b] = rB

        def stage_b(b):
            """Q projection, attention, PV, normalize, output."""
            xT, kT, vt = xT_b[b], kT_b[b], vt_b[b]
            (rrow_t, mrow_t), rB = rows_b[b], rB_b[b]

            qnT = qpool.tile([128, DC, T], BF16, tag="qnT")
            for dc in range(DC):
                for hf in range(2):
                    qp = fq.tile([128, 512], F32, tag="fq")
                    for kc in range(DC):
                        nc.tensor.matmul(
                            qp,
                            lhsT=wq_sb[:, kc, dc * 128:(dc + 1) * 128],
                            rhs=xT[:, kc, hf * 512:(hf + 1) * 512],
                            start=(kc == 0), stop=False,
                        )
                    # mean fix: -colsum(Wq')[o] * mean[t] as a 1-row matmul
                    nc.tensor.matmul(
                        qp,
                        lhsT=nwqs_sb[:, dc * 128:(dc + 1) * 128],
                        rhs=mrow_t[:, hf * 4:(hf + 1) * 4, :],
                        start=False, stop=True,
                    )
                    if cq_sb is not None:
                        nc.vector.tensor_scalar_add(
                            out=qp, in0=qp, scalar1=cq_sb[:, dc:dc + 1]
                        )
                    # q^T = qraw^T * rstd[t]  (deferred LN scale)
                    nc.vector.tensor_mul(
                        out=qnT[:, dc, hf * 512:(hf + 1) * 512],
                        in0=qp, in1=rB[:, hf, :],
                    )

            # S^T / P^T per (head pair, T half); head h at kT chunk h//2,
            # partition offset 64*(h%2); row-packed via tile_position into
            # one 2-bank PSUM tile, single Exp ACT over [77, 1024].
            pt_tiles = {}
            for hp in range(H // 2):
                for hf in range(2):
                    pt = ptpool.tile([N, 2, 512], BF16, tag="pt")
                    stt = stp.tile([N, 2, 512], F32, tag="st")
                    for sub in range(2):
                        po = 64 * sub
                        nc.tensor.matmul(
                            stt[:, sub, :],
                            lhsT=kT[po:po + 64, hp, :],
                            rhs=qnT[po:po + 64, hp, hf * 512:(hf + 1) * 512],
                            start=True, stop=True,
                            tile_position=(po, 0),
                        )
                    nc.scalar.activation(
                        out=pt, in_=stt,
                        func=mybir.ActivationFunctionType.Exp,
                        scale=float(SCALE),
                    )
                    pt_tiles[(hp, hf)] = pt

            for hf in range(2):
                y_half = ypool.tile([128, 4, D], BF16, tag="y")
                for ci in range(4):
                    c = hf * 4 + ci
                    yp0 = yps.tile([128, 4, HD + 1], F32, tag="yp")
                    yp1 = yps.tile([128, 4, HD + 1], F32, tag="yp")
                    ypl = (yp0, yp1)
                    for h in range(H):
                        pt = pt_tiles[(h // 2, hf)]
                        nc.tensor.matmul(
                            ypl[h // 4][:, h % 4, :],
                            lhsT=pt[:, h % 2, ci * 128:(ci + 1) * 128],
                            rhs=vt[:, h, :],
                            start=True, stop=True,
                        )
                    for j in range(2):
                        rs = stats.tile([128, 4], F32, tag="rs")
                        nc.vector.reciprocal(out=rs, in_=ypl[j][:, :, HD:HD + 1])
                        rs_ap = rs[:, :]
                        rs_b = bass.AP(
                            tensor=rs_ap.tensor, offset=rs_ap.offset,
                            ap=[rs_ap.ap[0], rs_ap.ap[1], [0, HD]],
                        )
                        nc.vector.tensor_mul(
                            out=y_half[:, ci].rearrange(
                                "p (j h d) -> p j h d", j=2, h=4
                            )[:, j],
                            in0=ypl[j][:, :, 0:HD],
                            in1=rs_b,
                        )
                nc.gpsimd.dma_start(
                    out=y[b, hf * 512:(hf + 1) * 512].rearrange(
                        "(c p) d -> p c d", p=128
                    ),
                    in_=y_half,
                )

        # Software pipeline: A(0) A(1) B(0) A(2) B(1) A(3) B(2) B(3)
        stage_a(0)
        stage_a(1)
        stage_b(0)
        stage_a(2)
        stage_b(1)
        stage_a(3)
        stage_b(2)
        stage_b(3)


_CACHE = {}
TRACE = False          # set True to capture an NTFF profile on core 0
LAST_RESULTS = None    # BassKernelResults of the most recent kernel() call


def _get_nc(key):
    if key not in _CACHE:
        _CACHE[key] = _build(*key)
    return _CACHE[key]


def kernel(x, xf, ln_g, ln_b, tln_g, tln_b, Wq, bq, Wk, bk, Wv, bv):
    x = np.asarray(x, np.float32)
    xf = np.asarray(xf, np.float32)
    # Fold layernorm affine + biases into the projections (f32 host math).
    wq_f = np.asarray(ln_g, np.float32)[:, None] * np.asarray(Wq, np.float32)
    cq = np.asarray(ln_b, np.float32) @ np.asarray(Wq, np.float32) + np.asarray(bq, np.float32)
    wk_f = np.asarray(tln_g, np.float32)[:, None] * np.asarray(Wk, np.float32)
    ck = np.asarray(tln_b, np.float32) @ np.asarray(Wk, np.float32) + np.asarray(bk, np.float32)
    wv_f = np.asarray(tln_g, np.float32)[:, None] * np.asarray(Wv, np.float32)
    cv = np.asarray(tln_b, np.float32) @ np.asarray(Wv, np.float32) + np.asarray(bv, np.float32)
    nwqs = -wq_f.sum(axis=0).reshape(1, D)

    has_cq = bool(np.any(cq != 0))
    has_ck = bool(np.any(ck != 0))
    has_cv = bool(np.any(cv != 0))
    nc = _get_nc((BPC, has_cq, has_ck, has_cv))

    bf = ml_dtypes.bfloat16
    wq_b = wq_f.astype(bf)
    wk_b = wk_f.astype(bf)
    wv_b = wv_f.astype(bf)
    x_b = x.astype(bf)
    xf_b = xf.astype(bf)

    in_maps = []
    for i in range(NCORES):
        m = {
            "xh": np.ascontiguousarray(x_b[i * BPC:(i + 1) * BPC]),
            "xfh": np.ascontiguousarray(xf_b[i * BPC:(i + 1) * BPC]),
            "wq": wq_b, "wk": wk_b, "wv": wv_b,
            "nwqs": nwqs.astype(bf),
        }
        if has_cq:
            m["cq"] = cq.reshape(1, D)
        if has_ck:
            m["ck"] = ck.reshape(1, D)
        if has_cv:
            m["cv"] = cv.reshape(1, D).astype(bf)
        in_maps.append(m)

    global LAST_RESULTS
    res = run_bass_kernel_spmd(
        nc, in_maps, core_ids=list(range(NCORES)), trace=TRACE
    )
    LAST_RESULTS = res
    out = np.concatenate([r["y"] for r in res.results], axis=0)
    return out.astype(np.float32)


# revision 31
# speedup vs baseline: 1.0794x; 1.0794x over previous
"""CrossAttention Trainium2 kernel.

Shapes (hardcoded from the problem spec):
  x  (32, 1024, 512) f32, xf (32, 77, 256) f32
  ln_g/ln_b (512,), tln_g/tln_b (256,)
  Wq (512,512), Wk (256,512), Wv (256,512), bq/bk/bv (512,)
  out y (32, 1024, 512) f32

Strategy:
  - Data-parallel over batch: 32 batches -> 8 cores x 4 batches. No collectives.
  - Host folds LayerNorm gamma/beta and biases into the projection weights
    (constant folding of parameters only), casts x/xf/weights to bf16.
  - x is read ONLY transposed, via whole-tensor DMA xbar loads (x^T lands as
    [128, dc, t]).  The x LayerNorm is computed entirely in row layout:
      mean row   m[1,T] = (1/512 ones)^T @ x^T          (PE matmul)
      E[x2] row  s[1,T] = (1/512 ones)^T @ (x^T * x^T)  (square on vector,
                                                         PE matmul)
      var = s - m^2 (gpsimd), rstd row r = exp(-.5 ln(var+eps)) (scalar)
    and the affine is deferred past the Q matmul:
      qraw^T = Wq'^T @ x^T - colsum(Wq') (x) m   (mean fix = 5th accumulating
                matmul with single-row lhsT/rhs)
      q^T    = qraw^T * rB    (rB = r broadcast down partitions by a tiny
                PE outer product; one tensor_tensor per Q tile)
  - Device per batch:
      xf: bn_stats layernorm -> PE transpose -> K^T and [V|1] projections
      S^T per head pair row-packed (tile_position) into a 2-bank PSUM tile,
      ONE Exp ACT per (head pair, T half) over [77,1024]
      y = P^T.T @ [V|1], softmax denominator in column 64; normalize on
      vector; y out per T-half on gpsimd.
  - All input DMAs are issued up front in ring order (scalar ring: xf +
    weights; sync ring: x^T xbar loads; gpsimd ring: y stores only) to avoid
    descriptor/semaphore-recycling stalls between rings.
"""

import numpy as np
import ml_dtypes

import concourse.bass as bass
import concourse.bacc as bacc
import concourse.mybir as mybir
import concourse.tile as tile
from concourse.bass_utils import run_bass_kernel_spmd
from concourse.masks import make_identity

B, T, D, N, L, H = 32, 1024, 512, 77, 256, 8
HD = D // H           # 64
NCORES = 8
BPC = B // NCORES     # 4 batches per core
EPS = 1e-5
SCALE = 1.0 / np.sqrt(HD)  # 0.125

BF16 = mybir.dt.bfloat16
F32 = mybir.dt.float32

TC = T // 128         # 8 T-chunks per batch
DC = D // 128         # 4 D-chunks
LC = L // 128         # 2 L-chunks


class _Bacc(bacc.Bacc):
    """Bacc whose ACT-table chooser only finds Exp/Ln in the combined
    natural_log_exp_and_others set, so the kernel needs one table load
    instead of ping-ponging between exp_and_others and the ln set."""

    def insert_act_table_loads(self):
        import bass_rust as _br
        from concourse.hw_specs import get_activation_tables

        has_activation = any(
            isinstance(i, mybir.InstActivation)
            for blk in self.main_func.blocks
            for i in blk.instructions
        )
        if not has_activation:
            return
        pair = {
            mybir.ActivationFunctionType.Exp,
            mybir.ActivationFunctionType.Ln,
        }
        tables = []
        for name, fns in get_activation_tables(self.m.arch).items():
            if name != "natural_log_exp_and_others":
                fns = fns - pair
            tables.append((name, fns))
        _br.insert_act_table_loads(self, tables)


def _build(bpc=BPC, has_cq=False, has_ck=False, has_cv=False):
    nc = _Bacc("TRN2", target_bir_lowering=False, debug=False)

    xh = nc.dram_tensor("xh", (bpc, T, D), BF16, kind="ExternalInput")
    xfh = nc.dram_tensor("xfh", (bpc, N, L), BF16, kind="ExternalInput")
    wq = nc.dram_tensor("wq", (D, D), BF16, kind="ExternalInput")
    wk = nc.dram_tensor("wk", (L, D), BF16, kind="ExternalInput")
    wv = nc.dram_tensor("wv", (L, D), BF16, kind="ExternalInput")
    nwqs = nc.dram_tensor("nwqs", (1, D), BF16, kind="ExternalInput")
    cq_d = nc.dram_tensor("cq", (1, D), F32, kind="ExternalInput") if has_cq else None
    ck_d = nc.dram_tensor("ck", (1, D), F32, kind="ExternalInput") if has_ck else None
    cv_d = nc.dram_tensor("cv", (1, D), BF16, kind="ExternalInput") if has_cv else None
    y = nc.dram_tensor("y", (bpc, T, D), BF16, kind="ExternalOutput")

    with tile.TileContext(nc) as tc:
        _trace(tc, bpc, xh, xfh, wq, wk, wv, nwqs, cq_d, ck_d, cv_d, y)
    nc.compile()
    return nc


def _trace(tc, bpc, xh, xfh, wq, wk, wv, nwqs, cq_d, ck_d, cv_d, y):
    nc = tc.nc
    from contextlib import ExitStack

    ctx = ExitStack()
    with ctx:
        consts = ctx.enter_context(tc.tile_pool(name="consts", bufs=1))
        stats = ctx.enter_context(tc.tile_pool(name="stats", bufs=8))
        rowsp = ctx.enter_context(tc.tile_pool(name="rowsp", bufs=3))
        rbpool = ctx.enter_context(tc.tile_pool(name="rbpool", bufs=3))
        xfpool = ctx.enter_context(tc.tile_pool(name="xfpool", bufs=6))
        kvpool = ctx.enter_context(tc.tile_pool(name="kvpool", bufs=6))
        xtpool = ctx.enter_context(tc.tile_pool(name="xtpool", bufs=4))
        xsqpool = ctx.enter_context(tc.tile_pool(name="xsqpool", bufs=2))
        qpool = ctx.enter_context(tc.tile_pool(name="qpool", bufs=2))
        ptpool = ctx.enter_context(tc.tile_pool(name="ptpool", bufs=10))
        ypool = ctx.enter_context(tc.tile_pool(name="ypool", bufs=4))
        # PSUM pools: 8 banks total. fq 2x1 + st 2x2 + yps 2x1 = 8.
        fq = ctx.enter_context(tc.tile_pool(name="fq", bufs=2, space="PSUM"))
        stp = ctx.enter_context(tc.tile_pool(name="stp", bufs=2, space="PSUM"))
        yps = ctx.enter_context(tc.tile_pool(name="yps", bufs=2, space="PSUM"))

        # ---- all input DMAs up front, grouped by ring ----
        # scalar hwdge ring: xf (small, gates the first PE work) then weights
        xf_ts = []
        for b in range(bpc):
            xf_t = xfpool.tile([N, L], BF16, tag="xf", name="xf_t")
            nc.scalar.dma_start(out=xf_t, in_=xfh[b])
            xf_ts.append(xf_t)
        wk_sb = consts.tile([128, LC, D], BF16, tag="wk")
        nc.scalar.dma_start(out=wk_sb, in_=wk.rearrange("(c p) d -> p c d", p=128))
        wv_sb = consts.tile([128, LC, D], BF16, tag="wv")
        nc.scalar.dma_start(out=wv_sb, in_=wv.rearrange("(c p) d -> p c d", p=128))
        wq_sb = consts.tile([128, DC, D], BF16, tag="wq")
        nc.scalar.dma_start(out=wq_sb, in_=wq.rearrange("(c p) d -> p c d", p=128))
        nwqs_sb = consts.tile([1, D], BF16, tag="nwqs")
        nc.scalar.dma_start(out=nwqs_sb, in_=nwqs[:, :])

        # sync ring: whole-tensor xbar transposes (xT[p, dc, t] = x[t, dc*128+p]).
        # b0 in T-halves so its first-half stats can start sooner.
        xT_b = {}
        for b in range(bpc):
            xT = xtpool.tile([128, DC, T], BF16, tag="xT", name="xT")
            if b == 0:
                for hf in range(2):
                    nc.sync.dma_start(
                        out=xT[:, :, hf * 512:(hf + 1) * 512],
                        in_=xh[b, hf * 512:(hf + 1) * 512],
                        transpose=True,
                    )
            else:
                nc.sync.dma_start(out=xT, in_=xh[b], transpose=True)
            xT_b[b] = xT

        # ---- constants ----
        eps_t = consts.tile([128, 1], F32, tag="eps")
        nc.vector.memset(eps_t, EPS)
        ident = consts.tile([128, 128], BF16, tag="ident")
        make_identity(nc, ident)
        ones1 = consts.tile([1, 128], BF16, tag="ones1")
        nc.vector.memset(ones1, 1.0)
        inv512 = consts.tile([128, 1], BF16, tag="inv512")
        nc.vector.memset(inv512, 1.0 / 512.0)
        cq_sb = ck_sb = cv_sb = None
        if cq_d is not None:
            cq_sb = consts.tile([128, DC], F32, tag="cq")  # [dout_part, chunk]
            nc.gpsimd.dma_start(
                out=cq_sb, in_=cq_d.rearrange("o (c p) -> (o p) c", p=128)
            )
        if ck_d is not None:
            ck_sb = consts.tile([128, DC], F32, tag="ck")
            nc.gpsimd.dma_start(
                out=ck_sb, in_=ck_d.rearrange("o (c p) -> (o p) c", p=128)
            )
        if cv_d is not None:
            cv_sb = consts.tile([1, D], BF16, tag="cv")
            nc.gpsimd.dma_start(out=cv_sb, in_=cv_d)
            ones_row = consts.tile([1, N], BF16, tag="ones_row")
            nc.vector.memset(ones_row, 1.0)

        kT_b, vt_b, rows_b, rB_b = {}, {}, {}, {}

        def stage_a(b):
            """xf path (K^T, [V|1]) + x row-layout stats (m_row, r_row)."""
            xf_t = xf_ts[b]
            st6 = stats.tile([N, 6], F32, tag="fst6")
            nc.vector.bn_stats(out=st6, in_=xf_t)
            mv_f = stats.tile([N, 2], F32, tag="fmv")
            nc.vector.bn_aggr(out=mv_f, in_=st6)
            # rstd = exp(-0.5*ln(var+eps)): Ln/Exp share one ACT table set.
            rstd_f = stats.tile([N, 1], F32, tag="frstd")
            nc.scalar.activation(
                out=rstd_f, in_=mv_f[:, 1:2],
                func=mybir.ActivationFunctionType.Ln,
                bias=eps_t[:N], scale=1.0,
            )
            nc.scalar.activation(
                out=rstd_f, in_=rstd_f,
                func=mybir.ActivationFunctionType.Exp, scale=-0.5,
            )
            xfn = xfpool.tile([N, L], BF16, tag="xfn")
            nc.vector.tensor_scalar(
                out=xfn, in0=xf_t,
                scalar1=mv_f[:, 0:1], scalar2=rstd_f,
                op0=mybir.AluOpType.subtract, op1=mybir.AluOpType.mult,
            )
            xfnT = xfpool.tile([128, LC, N], BF16, tag="xfnT")
            for c in range(LC):
                tps = fq.tile([128, N], BF16, tag="fq")
                nc.tensor.transpose(
                    out=tps, in_=xfn[:, c * 128:(c + 1) * 128], identity=ident[:N, :N]
                )
                nc.vector.tensor_copy(out=xfnT[:, c, :], in_=tps)

            kT = kvpool.tile([128, DC, N], BF16, tag="kT")
            for dc in range(DC):
                kps = fq.tile([128, N], F32, tag="fq")
                for lc in range(LC):
                    nc.tensor.matmul(
                        kps,
                        lhsT=wk_sb[:, lc, dc * 128:(dc + 1) * 128],
                        rhs=xfnT[:, lc, :],
                        start=(lc == 0), stop=(lc == LC - 1),
                    )
                if ck_sb is not None:
                    nc.vector.tensor_scalar_add(
                        out=kps, in0=kps, scalar1=ck_sb[:, dc:dc + 1]
                    )
                nc.scalar.copy(out=kT[:, dc, :], in_=kps)
            kT_b[b] = kT

            vps = fq.tile([N, D], F32, tag="fq")
            for lc in range(LC):
                nc.tensor.matmul(
                    vps, lhsT=xfnT[:, lc, :], rhs=wv_sb[:, lc, :],
                    start=(lc == 0), stop=(lc == LC - 1 and cv_sb is None),
                )
            if cv_sb is not None:
                nc.tensor.matmul(vps, lhsT=ones_row, rhs=cv_sb, start=False, stop=True)
            vt = kvpool.tile([N, H, HD + 1], BF16, tag="vt")
            nc.vector.tensor_copy(
                out=vt[:, :, 0:HD], in_=vps.rearrange("n (h d) -> n h d", h=H)
            )
            nc.vector.memset(vt[:, :, HD:HD + 1], 1.0)
            vt_b[b] = vt

            # x row stats via PE: m_row = inv512^T @ xT, s_row = inv512^T @ xT^2
            xT = xT_b[b]
            mrow_sb = rowsp.tile([1, 2, 512], BF16, tag="mrow")
            srow_sb = rowsp.tile([1, 2, 512], F32, tag="srow")
            for hf in range(2):
                mps = fq.tile([1, 512], F32, tag="fq")
                for kc in range(DC):
                    nc.tensor.matmul(
                        mps, lhsT=inv512,
                        rhs=xT[:, kc, hf * 512:(hf + 1) * 512],
                        start=(kc == 0), stop=(kc == DC - 1),
                    )
                nc.scalar.copy(out=mrow_sb[:, hf, :], in_=mps)
            xsq = xsqpool.tile([128, DC, T], BF16, tag="xsq")
            for kc in range(DC):
                nc.vector.tensor_mul(
                    out=xsq[:, kc, :], in0=xT[:, kc, :], in1=xT[:, kc, :]
                )
            for hf in range(2):
                sps = fq.tile([1, 512], F32, tag="fq")
                for kc in range(DC):
                    nc.tensor.matmul(
                        sps, lhsT=inv512,
                        rhs=xsq[:, kc, hf * 512:(hf + 1) * 512],
                        start=(kc == 0), stop=(kc == DC - 1),
                    )
                nc.scalar.copy(out=srow_sb[:, hf, :], in_=sps)
            # var = s - m^2 (gpsimd, single-lane), rstd = exp(-.5 ln(var+eps))
            msq = rowsp.tile([1, 2, 512], F32, tag="msq")
            nc.gpsimd.tensor_mul(out=msq, in0=mrow_sb, in1=mrow_sb)
            var_t = rowsp.tile([1, 2, 512], F32, tag="var")
            nc.gpsimd.tensor_sub(out=var_t, in0=srow_sb, in1=msq)
            rrow_sb = rowsp.tile([1, 2, 512], BF16, tag="rrow")
            nc.scalar.activation(
                out=rrow_sb, in_=var_t,
                func=mybir.ActivationFunctionType.Ln,
                bias=eps_t[:1], scale=1.0,
            )
            nc.scalar.activation(
                out=rrow_sb, in_=rrow_sb,
                func=mybir.ActivationFunctionType.Exp, scale=-0.5,
            )
            rows_b[b] = (mrow_sb, rrow_sb)

        def stage_b(b):
            """Q projection, attention, PV, normalize, output."""
            xT, kT, vt = xT_b[b], kT_b[b], vt_b[b]
            mrow_sb, rrow_sb = rows_b[b]

            # rB[p, hf, t] = rstd[t] broadcast down partitions (outer product)
            rB = rbpool.tile([128, 2, 512], BF16, tag="rB")
            for hf in range(2):
                rbp = fq.tile([128, 512], F32, tag="fq")
                nc.tensor.matmul(
                    rbp, lhsT=ones1, rhs=rrow_sb[:, hf, :],
                    start=True, stop=True,
                )
                nc.scalar.copy(out=rB[:, hf, :], in_=rbp)

            qnT = qpool.tile([128, DC, T], BF16, tag="qnT")
            for dc in range(DC):
                for hf in range(2):
                    qp = fq.tile([128, 512], F32, tag="fq")
                    for kc in range(DC):
                        nc.tensor.matmul(
                            qp,
                            lhsT=wq_sb[:, kc, dc * 128:(dc + 1) * 128],
                            rhs=xT[:, kc, hf * 512:(hf + 1) * 512],
                            start=(kc == 0), stop=False,
                        )
                    # mean fix: -colsum(Wq')[o] * mean[t] as a 1-row matmul
                    nc.tensor.matmul(
                        qp,
                        lhsT=nwqs_sb[:, dc * 128:(dc + 1) * 128],
                        rhs=mrow_sb[:, hf, :],
                        start=False, stop=True,
                    )
                    if cq_sb is not None:
                        nc.vector.tensor_scalar_add(
                            out=qp, in0=qp, scalar1=cq_sb[:, dc:dc + 1]
                        )
                    # q^T = qraw^T * rstd[t]  (deferred LN scale)
                    nc.vector.tensor_mul(
                        out=qnT[:, dc, hf * 512:(hf + 1) * 512],
                        in0=qp, in1=rB[:, hf, :],
                    )

            # S^T / P^T per (head pair, T half); head h at kT chunk h//2,
            # partition offset 64*(h%2); row-packed via tile_position into
            # one 2-bank PSUM tile, single Exp ACT over [77, 1024].
            pt_tiles = {}
            for hp in range(H // 2):
                for hf in range(2):
                    pt = ptpool.tile([N, 2, 512], BF16, tag="pt")
                    stt = stp.tile([N, 2, 512], F32, tag="st")
                    for sub in range(2):
                        po = 64 * sub
                        nc.tensor.matmul(
                            stt[:, sub, :],
                            lhsT=kT[po:po + 64, hp, :],
                            rhs=qnT[po:po + 64, hp, hf * 512:(hf + 1) * 512],
                            start=True, stop=True,
                            tile_position=(po, 0),
                        )
                    nc.scalar.activation(
                        out=pt, in_=stt,
                        func=mybir.ActivationFunctionType.Exp,
                        scale=float(SCALE),
                    )
                    pt_tiles[(hp, hf)] = pt

            for hf in range(2):
                y_half = ypool.tile([128, 4, D], BF16, tag="y")
                for ci in range(4):
                    yp0 = yps.tile([128, 4, HD + 1], F32, tag="yp")
                    yp1 = yps.tile([128, 4, HD + 1], F32, tag="yp")
                    ypl = (yp0, yp1)
                    for h in range(H):
                        pt = pt_tiles[(h // 2, hf)]
                        nc.tensor.matmul(
                            ypl[h // 4][:, h % 4, :],
                            lhsT=pt[:, h % 2, ci * 128:(ci + 1) * 128],
                            rhs=vt[:, h, :],
                            start=True, stop=True,
                        )
                    for j in range(2):
                        rs = stats.tile([128, 4], F32, tag="rs")
                        nc.vector.reciprocal(out=rs, in_=ypl[j][:, :, HD:HD + 1])
                        rs_ap = rs[:, :]
                        rs_b = bass.AP(
                            tensor=rs_ap.tensor, offset=rs_ap.offset,
                            ap=[rs_ap.ap[0], rs_ap.ap[1], [0, HD]],
                        )
                        nc.vector.tensor_mul(
                            out=y_half[:, ci].rearrange(
                                "p (j h d) -> p j h d", j=2, h=4
                            )[:, j],
                            in0=ypl[j][:, :, 0:HD],
                            in1=rs_b,
                        )
                nc.gpsimd.dma_start(
                    out=y[b, hf * 512:(hf + 1) * 512].rearrange(
                        "(c p) d -> p c d", p=128
                    ),
                    in_=y_half,
                )

        # Software pipeline: A(0) A(1) B(0) A(2) B(1) A(3) B(2) B(3)
        stage_a(0)
        stage_a(1)
        stage_b(0)
        stage_a(2)
        stage_b(1)
        stage_a(3)
        stage_b(2)
        stage_b(3)


_CACHE = {}
TRACE = False          # set True to capture an NTFF profile on core 0
LAST_RESULTS = None    # BassKernelResults of the most recent kernel() call


def _get_nc(key):
    if key not in _CACHE:
        _CACHE[key] = _build(*key)
    return _CACHE[key]


def kernel(x, xf, ln_g, ln_b, tln_g, tln_b, Wq, bq, Wk, bk, Wv, bv):
    x = np.asarray(x, np.float32)
    xf = np.asarray(xf, np.float32)
    # Fold layernorm affine + biases into the projections (f32 host math).
    wq_f = np.asarray(ln_g, np.float32)[:, None] * np.asarray(Wq, np.float32)
    cq = np.asarray(ln_b, np.float32) @ np.asarray(Wq, np.float32) + np.asarray(bq, np.float32)
    wk_f = np.asarray(tln_g, np.float32)[:, None] * np.asarray(Wk, np.float32)
    ck = np.asarray(tln_b, np.float32) @ np.asarray(Wk, np.float32) + np.asarray(bk, np.float32)
    wv_f = np.asarray(tln_g, np.float32)[:, None] * np.asarray(Wv, np.float32)
    cv = np.asarray(tln_b, np.float32) @ np.asarray(Wv, np.float32) + np.asarray(bv, np.float32)
    nwqs = -wq_f.sum(axis=0).reshape(1, D)

    has_cq = bool(np.any(cq != 0))
    has_ck = bool(np.any(ck != 0))
    has_cv = bool(np.any(cv != 0))
    nc = _get_nc((BPC, has_cq, has_ck, has_cv))

    bf = ml_dtypes.bfloat16
    wq_b = wq_f.astype(bf)
    wk_b = wk_f.astype(bf)
    wv_b = wv_f.astype(bf)
    x_b = x.astype(bf)
    xf_b = xf.astype(bf)

    in_maps = []
    for i in range(NCORES):
        m = {
            "xh": np.ascontiguousarray(x_b[i * BPC:(i + 1) * BPC]),
            "xfh": np.ascontiguousarray(xf_b[i * BPC:(i + 1) * BPC]),
            "wq": wq_b, "wk": wk_b, "wv": wv_b,
            "nwqs": nwqs.astype(bf),
        }
        if has_cq:
            m["cq"] = cq.reshape(1, D)
        if has_ck:
            m["ck"] = ck.reshape(1, D)
        if has_cv:
            m["cv"] = cv.reshape(1, D).astype(bf)
        in_maps.append(m)

    global LAST_RESULTS
    res = run_bass_kernel_spmd(
        nc, in_maps, core_ids=list(range(NCORES)), trace=TRACE
    )
    LAST_RESULTS = res
    out = np.concatenate([r["y"] for r in res.results], axis=0)
    return out.astype(np.float32)
